# revision 23
# baseline (speedup 1.0000x reference)
"""GNN message-passing (PNA-style) Trainium2 Bass kernel, 8-core SPMD.

Self-contained: hardcodes problem shapes. kernel(**inputs) -> [4000, 1] f32.

Design: fp16 node features padded to 256B rows; per-edge h[src] gather via
super4 dma_gather (one SWDGE instruction per ~24-column chunk, idx = row>>2
int16) + DVE lane-select masks; sum/sumsq aggregation in fp32 via
tensor_reduce; 3-way split AllGather for compute/collective overlap.
"""
import sys
sys.path.insert(0, "/opt/trn_rl_repo")
import numpy as np

import concourse.bass as bass
import concourse.bacc as bacc
import concourse.tile as tile
from concourse import mybir
from concourse.bass_utils import run_bass_kernel_spmd
from concourse.masks import make_identity

fp32 = mybir.dt.float32
fp16 = mybir.dt.float16
i16 = mybir.dt.int16
AF = mybir.ActivationFunctionType
OP = mybir.AluOpType

# problem constants
N, E, G, D = 100000, 400000, 4000, 70
NC = 8
EW = 128            # padded row width (elements) = 256 B
ATOM_DIMS = np.array([119, 5, 12, 12, 10, 6, 6, 2, 2])
ATOM_OFFSETS = np.concatenate([[0], np.cumsum(ATOM_DIMS)[:-1]]).astype(np.int64)
DEG_HIST = np.array([0.0, 100.0, 400.0, 300.0, 200.0])
_bins = np.arange(len(DEG_HIST), dtype=np.float64)
AVG_LOG = float((np.log(_bins + 1.0) * DEG_HIST).sum() / DEG_HIST.sum())
BN_EPS = 1e-5
STD_EPS = 1e-5
P = 128
EA_PAD = -1000.0    # pad-slot ea value (message -> 0 after relu; fp16-safe)
BIG = 1000.0        # min-mask additive for tail padding
MAXCOLS = 24        # max gather-chunk columns (nb*d)
NBD_CAP = 24        # max nb*d per block
SPLITS = (0.50, 0.25)  # cumulative column fractions for 3-way cc split


def _insert_axis(ap_obj, pos, count):
    lst = [list(x) for x in ap_obj.ap]
    lst = lst[:pos] + [[0, count]] + lst[pos:]
    return bass.AP(ap_obj.tensor, ap_obj.offset, lst)


def _wrap16(flat):
    """int16 slot array -> [128, ceil(n/16)] wrapped (i -> (i%16, i//16)), x8 replicated."""
    n = len(flat)
    n16 = (n + 15) // 16
    a = np.zeros(n16 * 16, np.int16)
    a[:n] = flat
    w = a.reshape(n16, 16).T  # [16, n16]
    return np.tile(w, (8, 1)).copy()  # [128, n16]


def _prep(x, edge_index, edge_attr, batch, atom_emb):
    src = np.asarray(edge_index[0], np.int64)
    dst = np.asarray(edge_index[1], np.int64)
    batch = np.asarray(batch, np.int64)
    ea = np.asarray(edge_attr, np.float32)

    deg = np.bincount(dst, minlength=N)
    eorder = np.argsort(dst, kind="stable")
    rowptr = np.zeros(N + 1, np.int64)
    rowptr[1:] = np.cumsum(deg)

    # graph-aligned core node ranges
    gcnt = np.bincount(batch, minlength=G)
    gnode_start = np.zeros(G + 1, np.int64)
    gnode_start[1:] = np.cumsum(gcnt)
    core_gb = [0]
    for c in range(1, NC):
        target = c * N // NC
        gi = int(np.searchsorted(gnode_start, target))
        if gnode_start[gi] != target and gi > 0:
            gi = gi if abs(gnode_start[gi] - target) < abs(gnode_start[gi - 1] - target) else gi - 1
        core_gb.append(gi)
    core_gb.append(G)
    core_nodes = [(int(gnode_start[core_gb[c]]), int(gnode_start[core_gb[c + 1]])) for c in range(NC)]

    dmax = int(deg.max())
    exact_ds = list(range(0, min(dmax, 8) + 1))
    has_tail = dmax > 8
    dtail = dmax if has_tail else 0

    core_group_nodes = []
    for c in range(NC):
        n0, n1 = core_nodes[c]
        nd = deg[n0:n1]
        groups = [np.nonzero(nd == d)[0] + n0 for d in exact_ds]
        if has_tail:
            groups.append(np.nonzero(nd >= 9)[0] + n0)
        core_group_nodes.append(groups)

    ngroups = len(exact_ds) + (1 if has_tail else 0)
    dvals = exact_ds + ([dtail] if has_tail else [])
    NT_g = [max((len(core_group_nodes[c][g]) + P - 1) // P for c in range(NC)) for g in range(ngroups)]
    NT = 1 + sum(NT_g)          # +1 reserved front zero tile
    NB = NT * P
    NPAD = NC * NB
    assert NPAD % 4 == 0 and NPAD // 4 <= 32767, NPAD

    # proc order: tile 0 reserved (all pad), then groups
    proc = np.full((NC, NB), -1, np.int64)
    goff = []
    ti = 1
    for g in range(ngroups):
        goff.append(ti)
        ti += NT_g[g]
    for c in range(NC):
        for g in range(ngroups):
            nodes = core_group_nodes[c][g]
            off = goff[g] * P
            proc[c, off:off + len(nodes)] = nodes

    pos_of_node = np.full(N, -1, np.int64)   # local pos within owning core
    core_of_node = np.full(N, -1, np.int64)
    for c in range(NC):
        mask = proc[c] >= 0
        pos_of_node[proc[c][mask]] = np.nonzero(mask)[0]
        core_of_node[proc[c][mask]] = c
    assert (pos_of_node >= 0).sum() == N

    # blocks: per group, tiles chunked, nb*d <= NBD_CAP
    blocks = []  # (g, d, t0, nb)
    for g in range(ngroups):
        d = dvals[g]
        nb_max = 4 if d == 0 else max(1, min(4, NBD_CAP // d))
        for b0 in range(0, NT_g[g], nb_max):
            nb = min(nb_max, NT_g[g] - b0)
            blocks.append((g, d, goff[g] + b0, nb))

    # 3-way part boundaries at block boundaries (by column count)
    totcols = sum(d * nb for (_, d, _, nb) in blocks if d > 0)
    part_of_block = []
    cum = 0
    for (g, d, t0, nb) in blocks:
        frac = cum / max(totcols, 1)
        part_of_block.append(0 if frac < SPLITS[0] else (1 if frac < SPLITS[0] + SPLITS[1] else 2))
        cum += d * nb
    # part boundaries as tile indices: T1 = min tile of part1, T2 = min tile of part2
    T1 = min((b[2] for b, p in zip(blocks, part_of_block) if p >= 1), default=NT)
    T2 = min((b[2] for b, p in zip(blocks, part_of_block) if p >= 2), default=NT)
    # parts must be tile-contiguous: blocks are emitted group-major == tile order
    for b, p in zip(blocks, part_of_block):
        t0, nb = b[2], b[3]
        if p == 0:
            assert t0 + nb <= T1
        elif p == 1:
            assert T1 <= t0 and t0 + nb <= T2
        else:
            assert T2 <= t0
    H1, H2 = T1 * P, T2 * P

    def g2row(c, pos):
        if pos < H1:
            return c * H1 + pos
        if pos < H2:
            return NC * H1 + c * (H2 - H1) + (pos - H1)
        return NC * H2 + c * (NB - H2) + (pos - H2)

    # column layout + gather metadata
    ea_offsets = []   # (g, d, t0, nb, col, ea_off, bcol)
    col = 0
    ea_off = 0
    bcol = 0
    for (g, d, t0, nb) in blocks:
        if d == 0:
            continue
        ea_offsets.append((g, d, t0, nb, col, ea_off, bcol))
        col += nb * d
        ea_off += P * nb * d * D
        if has_tail and g == ngroups - 1:
            bcol += nb * d
    NIDX = col
    TOTEA = max(ea_off, 1)
    bigpad_cols = max(bcol, 1)

    srcrow = np.zeros((NC, P, max(NIDX, 1)), np.int64)   # g2 row per slot (0 for pad)
    lane_m = np.zeros((NC, 4, P, max(NIDX, 1)), np.float16)
    eaflat = np.zeros((NC, TOTEA), np.float16)
    bigpad = np.zeros((NC, P, bigpad_cols), np.float16)

    ea16 = ea.astype(np.float16)
    for c in range(NC):
        for (g, d, t0, nb, col0, eo, bc0) in ea_offsets:
            blk = np.full((P, nb * d, D), EA_PAD, np.float16)
            for i in range(nb):
                t = t0 + i
                for p in range(P):
                    node = proc[c, t * P + p]
                    if node < 0:
                        continue
                    nd = int(deg[node])
                    use = min(nd, d)
                    eids = eorder[rowptr[node]:rowptr[node] + use]
                    blk[p, i * d:i * d + use] = ea16[eids]
                    ss = src[eids]
                    rows = np.array([g2row(int(core_of_node[s]), int(pos_of_node[s])) for s in ss], np.int64)
                    srcrow[c, p, col0 + i * d:col0 + i * d + use] = rows
                    lane_m[c, rows & 3, p, col0 + i * d + np.arange(use)] = 1.0
                    if g == ngroups - 1 and has_tail and nd < d:
                        bigpad[c, p, bc0 + i * d + nd:bc0 + (i + 1) * d] = BIG
            eaflat[c, eo:eo + P * nb * d * D] = blk.reshape(-1)

    # idx16: slot i = col*128 + p -> srcrow >> 2
    idx16 = np.zeros((NC, 128, max(NIDX * 8, 1)), np.int16)
    for c in range(NC):
        flat = (srcrow[c].T.reshape(-1) >> 2).astype(np.int16)  # [NIDX*128] in (col, p) order
        idx16[c, :, :len(flat) // 16] = _wrap16(flat)[:, :len(flat) // 16]

    # statics [NC, 3, NT, P]: 1/deg_safe, amp, 1/amp
    statn = np.zeros((NC, 3, NT, P), np.float32)
    for c in range(NC):
        nodes = proc[c]
        dd = np.where(nodes >= 0, deg[np.clip(nodes, 0, N - 1)], 0).astype(np.float64)
        dsafe = np.maximum(dd, 1.0)
        amp = np.log(dsafe + 1.0) / AVG_LOG
        statn[c, 0] = (1.0 / dsafe).astype(np.float32).reshape(NT, P)
        statn[c, 1] = amp.astype(np.float32).reshape(NT, P)
        statn[c, 2] = (1.0 / amp).astype(np.float32).reshape(NT, P)

    # h0 (AtomEncoder), padded fp16, proc order + g2 full table
    xl = np.asarray(x, np.int64)
    emb = np.asarray(atom_emb, np.float32)
    h0_all = emb[xl + ATOM_OFFSETS[None, :]].sum(axis=1)
    h0_own = np.zeros((NC, NB, EW), np.float16)
    for c in range(NC):
        mask = proc[c] >= 0
        h0_own[c][mask, :D] = h0_all[proc[c][mask]].astype(np.float16)
    h0_full = np.zeros((NPAD, EW), np.float16)
    for c in range(NC):
        for pos0 in range(0, NB, P):
            rows = np.array([g2row(c, pos0 + p) for p in range(P)])
            h0_full[rows] = h0_own[c, pos0:pos0 + P]

    # pooling: per core graphs sorted by size desc; common tile grid
    core_graphs = []
    for c in range(NC):
        gids = np.arange(core_gb[c], core_gb[c + 1])
        order = np.argsort(-gcnt[gids], kind="stable")
        core_graphs.append(gids[order])
    NGT = max((len(cg) + P - 1) // P for cg in core_graphs)
    KG_t = []
    for t in range(NGT):
        m = 1
        for c in range(NC):
            cg = core_graphs[c]
            if t * P < len(cg):
                m = max(m, int(gcnt[cg[t * P]]))
        KG_t.append(m)
    npoolcols = sum(KG_t)
    poolpos = np.zeros((NC, P, npoolcols), np.int64)  # local pos; 0 = reserved zero row
    ginv = np.ones((NC, NGT, P), np.float32)
    pc = 0
    pool_cols = []
    for t in range(NGT):
        pool_cols.append(pc)
        for c in range(NC):
            cg = core_graphs[c]
            for p in range(P):
                if t * P + p >= len(cg):
                    continue
                gid = cg[t * P + p]
                sz = int(gcnt[gid])
                ginv[c, t, p] = 1.0 / max(sz, 1)
                if sz > 0:
                    nids = np.arange(gnode_start[gid], gnode_start[gid] + sz)
                    poolpos[c, p, pc:pc + sz] = pos_of_node[nids]
        pc += KG_t[t]
    poolidx16 = np.zeros((NC, 128, max(npoolcols * 8, 1)), np.int16)
    for c in range(NC):
        flat = poolpos[c].T.reshape(-1).astype(np.int16)
        poolidx16[c, :, :len(flat) // 16] = _wrap16(flat)[:, :len(flat) // 16]

    # gather chunks per part: pack d>0 blocks up to MAXCOLS columns
    chunksP = ([], [], [])
    d0P = ([], [], [])
    for bi, b in enumerate(blocks):
        (g, d, t0, nb) = b
        pnum = part_of_block[bi]
        if d == 0:
            d0P[pnum].append(b)
    for pnum in range(3):
        cur = []
        cc = 0
        for eb in ea_offsets:
            (g, d, t0, nb, col0, eo, bc0) = eb
            bpart = 0 if t0 < T1 else (1 if t0 < T2 else 2)
            if bpart != pnum:
                continue
            if cur and cc + nb * d > MAXCOLS:
                chunksP[pnum].append(cur)
                cur = []
                cc = 0
            cur.append(eb)
            cc += nb * d
        if cur:
            chunksP[pnum].append(cur)

    cfg = dict(NB=NB, NT=NT, NPAD=NPAD, NIDX=max(NIDX, 1), TOTEA=TOTEA,
               T1=T1, T2=T2, H1=H1, H2=H2,
               blocks=blocks, ea_offsets=ea_offsets, dvals=dvals, goff=goff,
               NT_g=NT_g, ngroups=ngroups, has_tail=has_tail, dtail=dtail,
               NGT=NGT, KG_t=KG_t, pool_cols=pool_cols, npoolcols=npoolcols,
               bigpad_cols=bigpad_cols, chunksP=chunksP, d0P=d0P)
    arrays = dict(idx16=idx16, lane_m=lane_m, eaflat=eaflat, statn=statn,
                  h0_own=h0_own, h0_full=h0_full, poolidx16=poolidx16, ginv=ginv,
                  bigpad=bigpad)
    asm = dict(core_graphs=core_graphs, core_gb=core_gb)
    return cfg, arrays, asm


def _prep_weights(post_w, post_b, bn_gamma, bn_beta, mlp_w1, mlp_b1, mlp_w2, mlp_b2, mlp_w3, mlp_b3):
    post_w = np.asarray(post_w, np.float32)
    post_b = np.asarray(post_b, np.float32)
    bn_gamma = np.asarray(bn_gamma, np.float32)
    bn_beta = np.asarray(bn_beta, np.float32)
    inv_std_bn = np.float32(1.0 / np.sqrt(1.0 + BN_EPS))
    wch = np.zeros((4, 3, P, 210), np.float16)
    for l in range(4):
        for ch in range(3):
            r0, r1 = ch * 128, min((ch + 1) * 128, 280)
            rows = r1 - r0
            for s in range(3):
                wch[l, ch, :rows, s * 70:(s + 1) * 70] = post_w[l, s * 280 + r0:s * 280 + r1, :].astype(np.float16)
    Grep = bn_gamma * inv_std_bn
    B2 = post_b * Grep + bn_beta
    w1 = np.asarray(mlp_w1, np.float32)
    w2 = np.asarray(mlp_w2, np.float32)
    w3 = np.asarray(mlp_w3, np.float32)
    reps = np.concatenate([Grep.ravel(), B2.ravel(), np.asarray(mlp_b1, np.float32),
                           np.asarray(mlp_b2, np.float32), np.asarray(mlp_b3, np.float32)]).astype(np.float32)
    reps = np.broadcast_to(reps, (P, reps.size)).copy()
    return dict(wch=wch, reps=reps, w1=w1, w2=w2, w3=w3)


def _build(cfg):
    NB, NT, NPAD, NIDX, TOTEA = cfg["NB"], cfg["NT"], cfg["NPAD"], cfg["NIDX"], cfg["TOTEA"]
    NGT, npoolcols = cfg["NGT"], cfg["npoolcols"]
    NREP = 4 * 70 + 4 * 70 + 35 + 17 + 1
    NI16 = max(NIDX * 8, 1)
    NPI16 = max(npoolcols * 8, 1)

    nc = bacc.Bacc("TRN2", target_bir_lowering=False, debug=False, num_devices=NC)
    h0_own = nc.dram_tensor("h0_own", [NB, EW], fp16, kind="ExternalInput").ap()
    h0_full = nc.dram_tensor("h0_full", [NPAD, EW], fp16, kind="ExternalInput").ap()
    idx16 = nc.dram_tensor("idx16", [128, NI16], i16, kind="ExternalInput").ap()
    lane_m = nc.dram_tensor("lane_m", [4, P, NIDX], fp16, kind="ExternalInput").ap()
    eaflat = nc.dram_tensor("eaflat", [TOTEA], fp16, kind="ExternalInput").ap()
    statn = nc.dram_tensor("statn", [P, 3 * NT], fp32, kind="ExternalInput").ap()
    bigpad_t = nc.dram_tensor("bigpad", [P, cfg["bigpad_cols"]], fp16, kind="ExternalInput").ap()
    poolidx16 = nc.dram_tensor("poolidx16", [128, NPI16], i16, kind="ExternalInput").ap()
    ginv = nc.dram_tensor("ginv", [NGT, P], fp32, kind="ExternalInput").ap()
    wch = nc.dram_tensor("wch", [4, 3, P, 210], fp16, kind="ExternalInput").ap()
    reps = nc.dram_tensor("reps", [P, NREP], fp32, kind="ExternalInput").ap()
    w1 = nc.dram_tensor("w1", [D, 35], fp32, kind="ExternalInput").ap()
    w2 = nc.dram_tensor("w2", [35, 17], fp32, kind="ExternalInput").ap()
    w3 = nc.dram_tensor("w3", [17, 1], fp32, kind="ExternalInput").ap()
    out_g = nc.dram_tensor("out_g", [NGT * P, 1], fp32, kind="ExternalOutput").ap()

    h_own = [None] + [nc.dram_tensor(f"h_own{l}", [NB, EW], fp16) for l in range(1, 5)]
    hbuf = [None] + [nc.dram_tensor(f"hbuf{l}", [NPAD, EW], fp16) for l in range(1, 4)]

    # persistent SBUF
    # per-gather-call idx tensors: dma_gather needs idxs_ap at tensor base
    # and num_idxs <= 1024 (HW limit) -> <= GCAP columns per call
    GCAP = 8
    chunk_idx_sb = {}
    for part in range(3):
        for ci, chunk in enumerate(cfg["chunksP"][part]):
            ncols = sum(nb * d for (_, d, _, nb, _, _, _) in chunk)
            for k in range((ncols + GCAP - 1) // GCAP):
                w = min(GCAP, ncols - k * GCAP)
                chunk_idx_sb[(part, ci, k)] = nc.alloc_sbuf_tensor(
                    f"cidx_{part}_{ci}_{k}", [128, w * 8], i16).ap()
    pool_idx_sb = {}
    for t in range(cfg["NGT"]):
        KG = cfg["KG_t"][t]
        for k in range((KG + GCAP - 1) // GCAP):
            w = min(GCAP, KG - k * GCAP)
            pool_idx_sb[(t, k)] = nc.alloc_sbuf_tensor(
                f"pidx_{t}_{k}", [128, w * 8], i16).ap()
    mask_sb = nc.alloc_sbuf_tensor("mask_sb", [P, 4 * NIDX], fp16).ap()
    statn_sb = nc.alloc_sbuf_tensor("statn_sb", [P, 3 * NT], fp32).ap()
    wch_sb = nc.alloc_sbuf_tensor("wch_sb", [P, 4 * 3 * 210], fp16).ap()
    reps_sb = nc.alloc_sbuf_tensor("reps_sb", [P, NREP], fp32).ap()
    w1_sb = nc.alloc_sbuf_tensor("w1_sb", [D, 35], fp32).ap()
    w2_sb = nc.alloc_sbuf_tensor("w2_sb", [35, 17], fp32).ap()
    w3_sb = nc.alloc_sbuf_tensor("w3_sb", [17, 1], fp32).ap()
    ident16 = nc.alloc_sbuf_tensor("ident16", [P, P], fp16).ap()
    ident32 = nc.alloc_sbuf_tensor("ident32", [P, P], fp32).ap()
    epsb = nc.alloc_sbuf_tensor("epsb", [P, 1], fp32).ap()
    zrow = nc.alloc_sbuf_tensor("zrow", [P, EW], fp16).ap()

    cc_sems = {(l, h): nc.alloc_semaphore(name=f"ccs{l}_{h}") for l in range(1, 4) for h in range(3)}

    H1, H2 = cfg["H1"], cfg["H2"]
    part_rows = [(0, H1), (H1, H2), (H2, NB)]
    part_out = [(0, NC * H1), (NC * H1, NC * H2), (NC * H2, NPAD)]

    # ---- segment 0 ----
    with tile.TileContext(nc) as tc:
        with tc.tile_pool(name="s0", bufs=2) as pool:
            for part in range(3):
                for ci, chunk in enumerate(cfg["chunksP"][part]):
                    col0 = chunk[0][4]
                    ncols = sum(nb * d for (_, d, _, nb, _, _, _) in chunk)
                    for k in range((ncols + GCAP - 1) // GCAP):
                        w = min(GCAP, ncols - k * GCAP)
                        c0 = col0 + k * GCAP
                        nc.sync.dma_start(out=chunk_idx_sb[(part, ci, k)][:, :],
                                          in_=idx16[:, c0 * 8:(c0 + w) * 8])
            for t in range(cfg["NGT"]):
                pc = cfg["pool_cols"][t]
                KG = cfg["KG_t"][t]
                for k in range((KG + GCAP - 1) // GCAP):
                    w = min(GCAP, KG - k * GCAP)
                    c0 = pc + k * GCAP
                    nc.sync.dma_start(out=pool_idx_sb[(t, k)][:, :],
                                      in_=poolidx16[:, c0 * 8:(c0 + w) * 8])
            nc.sync.dma_start(out=mask_sb[:].rearrange("p (j c) -> p j c", j=4),
                              in_=lane_m.rearrange("j p c -> p j c"))
            nc.sync.dma_start(out=statn_sb[:, :], in_=statn[:, :])
            nc.sync.dma_start(out=wch_sb[:].rearrange("p (l c f) -> p l c f", l=4, c=3),
                              in_=wch.rearrange("l c p f -> p l c f"))
            nc.sync.dma_start(out=reps_sb[:, :], in_=reps[:, :])
            nc.sync.dma_start(out=w1_sb[:, :], in_=w1[:, :])
            nc.sync.dma_start(out=w2_sb[:, :], in_=w2[:, :])
            nc.sync.dma_start(out=w3_sb[:, :], in_=w3[:, :])
            make_identity(nc, ident16[:])
            make_identity(nc, ident32[:])
            nc.vector.memset(epsb[:], STD_EPS)
            nc.vector.memset(zrow[:], 0.0)
            # reserved front tile must be finite-zero in every h buffer:
            # it is allgathered and its rows sit inside gathered super4 rows
            for l in range(1, 5):
                nc.sync.dma_start(out=h_own[l].ap()[0:P, :], in_=zrow[:])

    def do_cc(l, part):
        a, b = part_rows[part]
        oa, ob = part_out[part]
        nc.gpsimd.collective_compute(
            "AllGather", OP.bypass,
            replica_groups=[list(range(NC))],
            ins=[h_own[l].ap()[a:b, :].opt()],
            outs=[hbuf[l].ap()[oa:ob, :].opt()],
        ).then_inc(cc_sems[(l, part)])

    def emit_chunk(chunk, l, hfull_t, pools, part, ci):
        """Gather a group of blocks (<=GCAP cols per call), then per-block compute."""
        (pool, gpool, spool, psp) = pools
        import os as _os
        col0 = chunk[0][4]
        ncols = sum(nb * d for (_, d, _, nb, _, _, _) in chunk)
        gt = gpool.tile([P, ncols * 512], fp16, tag="gt")
        if _os.environ.get("KERNEL_NOGATHER", "0") == "1":
            nc.vector.memset(gt[:], 0.0)
        else:
            for k in range((ncols + GCAP - 1) // GCAP):
                w = min(GCAP, ncols - k * GCAP)
                nc.gpsimd.dma_gather(
                    out_ap=gt[:, k * GCAP * 512:(k * GCAP + w) * 512].rearrange(
                        "p (c e) -> p c e", e=512),
                    in_ap=bass.AP(hfull_t, 0, [[512, NPAD // 4], [1, 512]]),
                    idxs_ap=chunk_idx_sb[(part, ci, k)][:, :],
                    num_idxs=w * P,
                    num_idxs_reg=w * P,
                    elem_size=512,
                )
        if _os.environ.get("KERNEL_NOCOMPUTE", "0") == "1":
            return
        for eb in chunk:
            emit_msg_block(eb, l, gt, col0, pool, spool, psp)

    def emit_msg_block(eb, l, gt, chunk_col0, pool, spool, psp):
        (g, d, t0, nb, col0, ea_off, bcol) = eb
        X = nb * d * D
        nbd = nb * d
        bc0 = col0 - chunk_col0
        hprev_own = h0_own if l == 1 else h_own[l - 1].ap()

        m = pool.tile([P, X], fp16, tag="m")
        tsel = pool.tile([P, X], fp16, tag="tsel")
        eat = pool.tile([P, X], fp16, tag="eat")
        hdst = pool.tile([P, nb * D], fp16, tag="hdst")
        nc.sync.dma_start(out=eat[:], in_=eaflat[ea_off:ea_off + P * X].rearrange("(p x) -> p x", p=P))
        nc.sync.dma_start(out=hdst[:],
                          in_=hprev_own[t0 * P:(t0 + nb) * P, 0:D].rearrange("(t p) f -> p t f", p=P))

        g4 = gt[:, bc0 * 512:(bc0 + nbd) * 512].rearrange("p (c j e) -> p c j e", j=4, e=128)
        m3 = m[:].rearrange("p (c f) -> p c f", f=D)
        t3 = tsel[:].rearrange("p (c f) -> p c f", f=D)
        mv = mask_sb[:].rearrange("p (j c) -> p j c", j=4)
        for j in range(4):
            mj = _insert_axis(mv[:, j, col0:col0 + nbd], 2, D)
            dstv = m3 if j == 0 else t3
            nc.vector.tensor_tensor(out=dstv, in0=g4[:, :, j, 0:D], in1=mj, op=OP.mult)
            if j > 0:
                nc.vector.tensor_tensor(out=m3, in0=m3, in1=t3, op=OP.add)
        # m += ea ; m += h_dst ; relu
        nc.vector.tensor_tensor(out=m[:], in0=m[:], in1=eat[:], op=OP.add)
        mt = m[:].rearrange("p (t j f) -> p t j f", t=nb, j=d)
        hdst_b = _insert_axis(hdst[:].rearrange("p (t f) -> p t f", t=nb), 2, d)
        nc.vector.tensor_tensor(out=mt, in0=mt, in1=hdst_b, op=OP.add)
        nc.scalar.activation(out=m[:], in_=m[:], func=AF.Relu)

        agg = spool.tile([P, nb * 280], fp16, tag="agg")
        a3 = agg[:].rearrange("p (t f) -> p t f", t=nb)
        is_tail = cfg["has_tail"] and g == cfg["ngroups"] - 1
        mr = m[:].rearrange("p (t j f) -> p t f j", t=nb, j=d)
        if d == 1:
            nc.vector.tensor_copy(out=a3[:, :, 70:140], in_=m3.rearrange("p t f -> p t f"))
            nc.vector.tensor_copy(out=a3[:, :, 140:210], in_=m3)
            s32 = spool.tile([P, nb * D], fp32, tag="s32")
            nc.vector.tensor_copy(out=s32[:], in_=m[:])
            s2 = None
        else:
            # min (tail: masked), max
            if is_tail:
                bp = pool.tile([P, nbd], fp16, tag="bp")
                nc.sync.dma_start(out=bp[:], in_=bigpad_t[:, bcol:bcol + nbd])
                mm = pool.tile([P, X], fp16, tag="mm")
                m4 = mm[:].rearrange("p (t j f) -> p t j f", t=nb, j=d)
                bp_b = _insert_axis(bp[:].rearrange("p (t j) -> p t j", t=nb), 3, D)
                nc.vector.tensor_tensor(out=m4, in0=mt, in1=bp_b, op=OP.add)
                nc.vector.tensor_reduce(out=a3[:, :, 70:140],
                                        in_=mm[:].rearrange("p (t j f) -> p t f j", t=nb, j=d),
                                        op=OP.min, axis=mybir.AxisListType.X)
            else:
                nc.vector.tensor_reduce(out=a3[:, :, 70:140], in_=mr, op=OP.min, axis=mybir.AxisListType.X)
            nc.vector.tensor_reduce(out=a3[:, :, 140:210], in_=mr, op=OP.max, axis=mybir.AxisListType.X)
            s32 = spool.tile([P, nb * D], fp32, tag="s32")
            nc.vector.tensor_reduce(out=s32[:].rearrange("p (t f) -> p t f", t=nb),
                                    in_=mr, op=OP.add, axis=mybir.AxisListType.X)
            msq = pool.tile([P, X], fp32, tag="msq")
            nc.scalar.activation(out=msq[:], in_=m[:], func=AF.Square)
            s2 = spool.tile([P, nb * D], fp32, tag="s2")
            nc.vector.tensor_reduce(out=s2[:].rearrange("p (t f) -> p t f", t=nb),
                                    in_=msq[:].rearrange("p (t j f) -> p t f j", t=nb, j=d),
                                    op=OP.add, axis=mybir.AxisListType.X)
        _stage2(nc, pool, spool, psp, cfg, statn_sb, a3, s32, s2,
                t0, nb, l, wch_sb, reps_sb, ident16, epsb, hdst, h_own[l].ap(), d)

    def emit_d0_block(blk, l, pool, spool, psp):
        (g, d, t0, nb) = blk
        hprev_own = h0_own if l == 1 else h_own[l - 1].ap()
        hdst = pool.tile([P, nb * D], fp16, tag="hdst")
        nc.sync.dma_start(out=hdst[:],
                          in_=hprev_own[t0 * P:(t0 + nb) * P, 0:D].rearrange("(t p) f -> p t f", p=P))
        agg = spool.tile([P, nb * 280], fp16, tag="agg")
        nc.vector.memset(agg[:], 0.0)
        a3 = agg[:].rearrange("p (t f) -> p t f", t=nb)
        _stage2(nc, pool, spool, psp, cfg, statn_sb, a3, None, None,
                t0, nb, l, wch_sb, reps_sb, ident16, epsb, hdst, h_own[l].ap(), d)

    # ---- layers ----
    import os as _os
    MAXL = int(_os.environ.get("KERNEL_MAXL", "4"))
    NOCC = _os.environ.get("KERNEL_NOCC", "0") == "1"
    for l in range(1, MAXL + 1):
        hfull_t = h0_full.tensor if l == 1 else hbuf[l - 1]
        if l >= 2 and not NOCC:
            for part in range(3):
                nc.gpsimd.wait_ge(cc_sems[(l - 1, part)], 1)
        for part in range(3):
            with tile.TileContext(nc) as tc:
                with tc.tile_pool(name=f"L{l}p{part}", bufs=2) as pool, \
                     tc.tile_pool(name=f"Lg{l}p{part}", bufs=2) as gpool, \
                     tc.tile_pool(name=f"Ls{l}p{part}", bufs=2) as spool, \
                     tc.tile_pool(name=f"Lp{l}p{part}", bufs=2, space="PSUM") as psp:
                    pools = (pool, gpool, spool, psp)
                    for ci, chunk in enumerate(cfg["chunksP"][part]):
                        emit_chunk(chunk, l, hfull_t, pools, part, ci)
                    for blk in cfg["d0P"][part]:
                        emit_d0_block(blk, l, pool, spool, psp)
                    if l == 4 and part == 2:
                        _pooling(nc, pool, spool, psp, cfg, pool_idx_sb, ginv, h_own[4],
                                 w1_sb, w2_sb, w3_sb, reps_sb, ident32, out_g)
            if l < 4 and not NOCC:
                do_cc(l, part)

    nc.compile()
    return nc


def _stage2(nc, pool, spool, psp, cfg, statn_sb, a3, s32, s2,
            t0, nb, l, wch_sb, reps_sb, ident16, epsb, hdst, hout, d):
    NT = cfg["NT"]
    sv = statn_sb[:].rearrange("p (k t) -> p k t", k=3)
    invc_b = _insert_axis(sv[:, 0, t0:t0 + nb], 2, 70)
    amp_b = _insert_axis(sv[:, 1, t0:t0 + nb], 2, 70)
    iamp_b = _insert_axis(sv[:, 2, t0:t0 + nb], 2, 70)

    if d == 0:
        # agg all zero; std = sqrt(eps)
        nc.scalar.activation(out=a3[:, :, 210:280], in_=a3[:, :, 0:70], func=AF.Sqrt, bias=epsb[:])
    elif d == 1:
        # mean=min=max=m (copied by caller); var = 0 exactly -> std const
        s3 = s32[:].rearrange("p (t f) -> p t f", t=nb)
        nc.vector.tensor_tensor(out=a3[:, :, 0:70], in0=s3, in1=invc_b, op=OP.mult)
        nc.vector.memset(a3[:, :, 210:280], float(np.sqrt(STD_EPS)))
    else:
        s3 = s32[:].rearrange("p (t f) -> p t f", t=nb)
        s23 = s2[:].rearrange("p (t f) -> p t f", t=nb)
        mean32 = spool.tile([P, nb * 70], fp32, tag="mean32")
        me3 = mean32[:].rearrange("p (t f) -> p t f", t=nb)
        nc.vector.tensor_tensor(out=me3, in0=s3, in1=invc_b, op=OP.mult)
        nc.vector.tensor_copy(out=a3[:, :, 0:70], in_=me3)
        u = spool.tile([P, nb * 70], fp32, tag="u")
        u3 = u[:].rearrange("p (t f) -> p t f", t=nb)
        nc.vector.tensor_tensor(out=u3, in0=s23, in1=invc_b, op=OP.mult)
        v = spool.tile([P, nb * 70], fp32, tag="v")
        v3 = v[:].rearrange("p (t f) -> p t f", t=nb)
        nc.vector.tensor_tensor(out=v3, in0=me3, in1=me3, op=OP.mult)
        nc.vector.tensor_tensor(out=u3, in0=u3, in1=v3, op=OP.subtract)
        nc.scalar.activation(out=u[:], in_=u[:], func=AF.Relu)
        nc.scalar.activation(out=a3[:, :, 210:280], in_=u3, func=AF.Sqrt, bias=epsb[:])

    # scaled copies: agg*amp at 280.., agg/amp handled via weights? no - baseline folds
    # post matmul per tile: psmm[:, 0:70]=A, 70:140=B(amp), 140:210=C(iamp)
    sabc = spool.tile([P, nb * 210], fp32, tag="sabc")
    for i in range(nb):
        aggT = pool.tile([P, P], fp16, tag="aggT")
        psmm = psp.tile([P, 210], fp32, space="PSUM", tag="psmm")
        for ch in range(3):
            rows = 128 if ch < 2 else 24
            psT = psp.tile([P, P], fp16, space="PSUM", tag="psT")
            nc.tensor.transpose(out=psT[:rows, :],
                                in_=a3[:, i:i + 1, ch * 128:ch * 128 + rows].rearrange("p t f -> p (t f)"),
                                identity=ident16[:])
            nc.vector.tensor_copy(out=aggT[:rows, :], in_=psT[:rows, :])
            nc.tensor.matmul(out=psmm[:, :], lhsT=aggT[:rows, :],
                             rhs=wch_sb[:].rearrange("p (l c f) -> p l c f", l=4, c=3)[:rows, l - 1, ch, :],
                             start=(ch == 0), stop=(ch == 2))
        nc.vector.tensor_copy(out=sabc[:, i * 210:(i + 1) * 210], in_=psmm[:, :])

    sA = sabc[:].rearrange("p (t f) -> p t f", t=nb)[:, :, 0:70]
    sB = sabc[:].rearrange("p (t f) -> p t f", t=nb)[:, :, 70:140]
    sC = sabc[:].rearrange("p (t f) -> p t f", t=nb)[:, :, 140:210]
    hn = pool.tile([P, nb * 70], fp16, tag="hn")
    hn3 = hn[:].rearrange("p (t f) -> p t f", t=nb)
    tmp = pool.tile([P, nb * 70], fp32, tag="tmp")
    tmp3 = tmp[:].rearrange("p (t f) -> p t f", t=nb)
    nc.vector.tensor_tensor(out=tmp3, in0=sB, in1=amp_b, op=OP.mult)
    nc.vector.tensor_tensor(out=tmp3, in0=tmp3, in1=sA, op=OP.add)
    tmp2 = pool.tile([P, nb * 70], fp32, tag="tmp2")
    tmp23 = tmp2[:].rearrange("p (t f) -> p t f", t=nb)
    nc.vector.tensor_tensor(out=tmp23, in0=sC, in1=iamp_b, op=OP.mult)
    nc.vector.tensor_tensor(out=tmp3, in0=tmp3, in1=tmp23, op=OP.add)
    # BN affine + relu + residual
    Grep_b = _insert_axis(reps_sb[:, (l - 1) * 70:l * 70], 1, nb)
    B2_b = _insert_axis(reps_sb[:, 280 + (l - 1) * 70:280 + l * 70], 1, nb)
    nc.vector.tensor_tensor(out=tmp3, in0=tmp3, in1=Grep_b, op=OP.mult)
    nc.vector.tensor_tensor(out=hn3, in0=tmp3, in1=B2_b, op=OP.add)
    nc.scalar.activation(out=hn[:], in_=hn[:], func=AF.Relu)
    nc.vector.tensor_tensor(out=hn[:], in0=hn[:], in1=hdst[:], op=OP.add)
    nc.sync.dma_start(out=hout[t0 * P:(t0 + nb) * P, 0:D].rearrange("(t p) f -> p t f", p=P),
                      in_=hn[:].rearrange("p (t f) -> p t f", t=nb))


def _pooling(nc, pool, spool, psp, cfg, pool_idx_sb, ginv, h4, w1_sb, w2_sb, w3_sb, reps_sb, ident32, out_g):
    boff = 560
    GCAP = 8
    for t in range(cfg["NGT"]):
        KG = cfg["KG_t"][t]
        pg = pool.tile([P, KG * 128], fp16, tag="pg")
        for k in range((KG + GCAP - 1) // GCAP):
            w = min(GCAP, KG - k * GCAP)
            nc.gpsimd.dma_gather(
                out_ap=pg[:, k * GCAP * 128:(k * GCAP + w) * 128].rearrange(
                    "p (c e) -> p c e", e=128),
                in_ap=h4.ap()[:, :],
                idxs_ap=pool_idx_sb[(t, k)][:, :],
                num_idxs=w * P,
                num_idxs_reg=w * P,
                elem_size=128,
            )
        pgv = pg[:].rearrange("p (c e) -> p c e", e=128)
        gsum = pool.tile([P, D], fp32, tag="gsum")
        nc.vector.tensor_reduce(out=gsum[:],
                                in_=pgv[:, :, 0:D].rearrange("p c f -> p f c"),
                                op=OP.add, axis=mybir.AxisListType.X)
        gv = pool.tile([P, 1], fp32, tag="gv")
        nc.sync.dma_start(out=gv[:], in_=ginv[t:t + 1, :].rearrange("o p -> p o"))
        nc.vector.tensor_scalar_mul(gsum[:], gsum[:], gv[:])
        psT = psp.tile([P, P], fp32, space="PSUM", tag="psT")
        nc.tensor.transpose(out=psT[:D, :], in_=gsum[:], identity=ident32[:])
        gT = pool.tile([D, P], fp32, tag="gT")
        nc.vector.tensor_copy(out=gT[:], in_=psT[:D, :])
        ps1 = psp.tile([P, 35], fp32, space="PSUM", tag="psmm")
        nc.tensor.matmul(out=ps1[:], lhsT=gT[:], rhs=w1_sb[:, :], start=True, stop=True)
        y1 = pool.tile([P, 35], fp32, tag="y1")
        nc.vector.tensor_tensor(out=y1[:], in0=ps1[:], in1=reps_sb[:, boff:boff + 35], op=OP.add)
        nc.scalar.activation(out=y1[:], in_=y1[:], func=AF.Relu)
        psT2 = psp.tile([P, P], fp32, space="PSUM", tag="psT")
        nc.tensor.transpose(out=psT2[:35, :], in_=y1[:], identity=ident32[:])
        y1T = pool.tile([35, P], fp32, tag="y1T")
        nc.vector.tensor_copy(out=y1T[:], in_=psT2[:35, :])
        ps2 = psp.tile([P, 17], fp32, space="PSUM", tag="psmm")
        nc.tensor.matmul(out=ps2[:], lhsT=y1T[:], rhs=w2_sb[:, :], start=True, stop=True)
        y2 = pool.tile([P, 17], fp32, tag="y2")
        nc.vector.tensor_tensor(out=y2[:], in0=ps2[:], in1=reps_sb[:, boff + 35:boff + 52], op=OP.add)
        nc.scalar.activation(out=y2[:], in_=y2[:], func=AF.Relu)
        psT3 = psp.tile([P, P], fp32, space="PSUM", tag="psT")
        nc.tensor.transpose(out=psT3[:17, :], in_=y2[:], identity=ident32[:])
        y2T = pool.tile([17, P], fp32, tag="y2T")
        nc.vector.tensor_copy(out=y2T[:], in_=psT3[:17, :])
        ps3 = psp.tile([P, 1], fp32, space="PSUM", tag="psmm")
        nc.tensor.matmul(out=ps3[:], lhsT=y2T[:], rhs=w3_sb[:, :], start=True, stop=True)
        y3 = pool.tile([P, 1], fp32, tag="y3")
        nc.vector.tensor_tensor(out=y3[:], in0=ps3[:], in1=reps_sb[:, boff + 52:boff + 53], op=OP.add)
        nc.sync.dma_start(out=out_g[t * P:(t + 1) * P, :], in_=y3[:])


def kernel(x, edge_index, edge_attr, batch, atom_emb, post_w, post_b,
           bn_gamma, bn_beta, mlp_w1, mlp_b1, mlp_w2, mlp_b2, mlp_w3, mlp_b3):
    cfg, arrays, asm = _prep(x, edge_index, edge_attr, batch, atom_emb)
    wd = _prep_weights(post_w, post_b, bn_gamma, bn_beta, mlp_w1, mlp_b1,
                       mlp_w2, mlp_b2, mlp_w3, mlp_b3)
    nc = _build(cfg)

    in_maps = []
    for c in range(NC):
        in_maps.append({
            "h0_own": arrays["h0_own"][c],
            "h0_full": arrays["h0_full"],
            "idx16": arrays["idx16"][c],
            "lane_m": arrays["lane_m"][c],
            "eaflat": arrays["eaflat"][c],
            "statn": arrays["statn"][c].transpose(2, 0, 1).reshape(P, -1).copy(),
            "bigpad": arrays["bigpad"][c],
            "poolidx16": arrays["poolidx16"][c],
            "ginv": arrays["ginv"][c],
            "wch": wd["wch"],
            "reps": wd["reps"],
            "w1": wd["w1"],
            "w2": wd["w2"],
            "w3": wd["w3"],
        })
    import os
    trace = os.environ.get("KERNEL_TRACE", "0") == "1"
    res = run_bass_kernel_spmd(nc, in_maps, core_ids=list(range(NC)), trace=trace)
    kernel.last_exec_time_ns = res.exec_time_ns
    y = np.zeros((G, 1), np.float32)
    for c in range(NC):
        og = res.results[c]["out_g"]
        cg = asm["core_graphs"][c]
        y[cg] = og[:len(cg)]
    return y
